# revision 13
# baseline (speedup 1.0000x reference)
"""Trainium2 Bass kernel for Transformer-XL style relative-position attention.

Problem: S=512, B=2, D=512, DQK=64, H=8, causal, OFFSET=0.
Sharding: one head per NeuronCore (8 heads / 8 cores); each core computes its
head's contribution to the output projection; host sums the 8 partials
(row-parallel tensor-parallel unshard).

Per core (head h), per batch b and 128-row query chunk c (causal-truncated to
W = 128*(c+1) key columns):
  qT,kT = Wqk @ x_b^T                       [64,512] (merged stationary)
  vT    = Wv @ x_b^T -> PE-transposed to v [m,64] tiles
  tableT_rev = P_h @ sincos_rev^T           [64,512] rel-pos table, reversed cols
  qrel_rev chunk = q @ tableT_rev[:, 512-W:] -> DRAM bounce (row pitch 512)
  position = strided "rel-shift" gather  pos[p,m] = qbuf[511*(128c+p)+511+m]
  causal mask via gpsimd affine_select(fill=-1e30) on the gathered tile
  logits = (content + position)*scale fused with row-max (tensor_tensor_reduce)
  exp on ScalarE with accum_out row-sums; E transposed 128x128 on PE
  ctx chunk = sum_j v_j^T @ ET(c,j); out chunk = (ctx^T @ WoT) * rinv (fused)

All matmuls run as float32r (fp32 bits, fast PE mode, ~1.6e-4 max rel err)
with fp32 PSUM accumulation.
"""

import math

import numpy as np

S, B, D = 512, 2, 512
DQK, H = 64, 8
P = 128
NCH = S // P  # 4 query-row chunks
KT = D // P   # 4 contraction tiles over D
SCALE = 1.0 / math.sqrt(float(D))
NEG = -1e30

_CACHE = {}


def _build_graph():
    import concourse.bass as bass
    import concourse.mybir as mybir
    import concourse.tile as tile
    from concourse import bacc

    F32 = mybir.dt.float32
    F32R = mybir.dt.float32r

    def r(ap):
        return ap.bitcast(F32R)

    nc = bacc.Bacc(None, target_bir_lowering=False, debug=True)

    xT_ext = nc.declare_dram_parameter("xT", [B, D, S], F32, isOutput=False)
    wqk_ext = nc.declare_dram_parameter("wqk", [D, 2 * DQK], F32, isOutput=False)
    wv_ext = nc.declare_dram_parameter("wv", [D, DQK], F32, isOutput=False)
    pp_ext = nc.declare_dram_parameter("pp", [D, DQK], F32, isOutput=False)
    wo_ext = nc.declare_dram_parameter("wo", [DQK, D], F32, isOutput=False)
    sct_ext = nc.declare_dram_parameter("sincosT", [D, S], F32, isOutput=False)
    id_ext = nc.declare_dram_parameter("ident", [P, P], F32, isOutput=False)
    out_ext = nc.declare_dram_parameter("out", [S, B, D], F32, isOutput=True)

    with tile.TileContext(nc) as tc:
        with tc.tile_pool(name="const", bufs=1) as const, \
             tc.tile_pool(name="proj", bufs=1) as proj, \
             tc.tile_pool(name="work", bufs=3) as work, \
             tc.tile_pool(name="outp", bufs=2) as outp, \
             tc.tile_pool(name="psA", bufs=2, space="PSUM") as psA, \
             tc.tile_pool(name="psB", bufs=1, space="PSUM") as psB, \
             tc.tile_pool(name="dram", bufs=1, space="DRAM") as dram:

            # ---- constant loads (SWDGE so they don't serialize behind sync DMAs) ----
            ident = const.tile([P, P], F32R)
            nc.gpsimd.dma_start(out=ident[:], in_=id_ext[:])
            wqk_sb = const.tile([P, KT, 2 * DQK], F32R)
            nc.gpsimd.dma_start(out=wqk_sb[:], in_=wqk_ext[:].rearrange("(k p) m -> p k m", p=P))
            wv_sb = const.tile([P, KT, DQK], F32R)
            nc.gpsimd.dma_start(out=wv_sb[:], in_=wv_ext[:].rearrange("(k p) m -> p k m", p=P))
            pp_sb = const.tile([P, KT, DQK], F32R)
            nc.gpsimd.dma_start(out=pp_sb[:], in_=pp_ext[:].rearrange("(k p) m -> p k m", p=P))
            wo_sb = const.tile([DQK, S], F32R)
            nc.gpsimd.dma_start(out=wo_sb[:], in_=wo_ext[:])
            sct_sb = const.tile([P, KT, S], F32R)
            nc.gpsimd.dma_start(out=sct_sb[:], in_=sct_ext[:].rearrange("(k p) m -> p k m", p=P))
            xT_sb = []
            for b in range(B):
                xb = const.tile([P, KT, S], F32R, tag=f"xT{b}")
                nc.gpsimd.dma_start(out=xb[:], in_=xT_ext[b].rearrange("(k p) m -> p k m", p=P))
                xT_sb.append(xb)

            # ---- rel-pos table: tableT_rev [64, 512] ----
            tb_ps = psA.tile([DQK, S], F32, tag="L")
            for k in range(KT):
                nc.tensor.matmul(tb_ps[:], r(pp_sb[:, k, :]), r(sct_sb[:, k, :]),
                                 start=(k == 0), stop=(k == KT - 1))
            tableT = proj.tile([DQK, S], F32R)
            nc.vector.tensor_copy(out=tableT[:], in_=tb_ps[:])

            # ---- projections ----
            qkT = []   # (qkT_tile [128,512] rows 0:64 = qT, kT_tile [64,512])
            v_sb = []  # [128, NCH, 64] m-tiles of v
            for b in range(B):
                qk_ps = psA.tile([P, S], F32, tag="L")
                for k in range(KT):
                    nc.tensor.matmul(qk_ps[:], r(wqk_sb[:, k, :]), r(xT_sb[b][:, k, :]),
                                     start=(k == 0), stop=(k == KT - 1))
                qx = proj.tile([P, S], F32R, tag=f"qkT{b}")
                nc.vector.tensor_copy(out=qx[:], in_=qk_ps[:])
                # rhs of a matmul must share lhsT's base partition: move kT to base 0
                kx = proj.tile([DQK, S], F32R, tag=f"kT{b}")
                nc.sync.dma_start(out=kx[:], in_=qx[DQK:2 * DQK, :])
                qkT.append((qx, kx))

                vT_ps = psA.tile([DQK, S], F32, tag="L")
                for k in range(KT):
                    nc.tensor.matmul(vT_ps[:], r(wv_sb[:, k, :]), r(xT_sb[b][:, k, :]),
                                     start=(k == 0), stop=(k == KT - 1))
                vT = work.tile([DQK, S], F32R, tag="vT")
                nc.vector.tensor_copy(out=vT[:], in_=vT_ps[:])
                vb = proj.tile([P, NCH, DQK], F32R, tag=f"v{b}")
                for j in range(NCH):
                    vt_ps = psB.tile([P, DQK], F32, tag="tr0")
                    nc.tensor.transpose(r(vt_ps[:]), r(vT[:, j * P:(j + 1) * P]),
                                        r(ident[0:DQK, 0:DQK]))
                    nc.vector.tensor_copy(out=vb[:, j, :], in_=vt_ps[:])
                v_sb.append(vb)

            # ---- attention: per (b, chunk) pipeline, causal-truncated widths ----
            # qbuf row pitch 768; cols [512, 640) pre-filled with NEG so the
            # rel-shift gather lands on -1e30 wherever m > n (causal mask for free)
            PT = 768
            qbuf = dram.tile([B, S, PT], F32)
            negt = work.tile([P, P], F32, tag="negt")
            nc.vector.memset(negt[:], NEG)
            for b in range(B):
                for c in range(NCH):
                    nc.sync.dma_start(out=qbuf[b][c * P:(c + 1) * P, S:S + P], in_=negt[:])
            for b in range(B):
                qb_ap = qbuf[b]
                qb_base = qb_ap.offset
                qT = qkT[b][0][0:DQK, :]
                kT = qkT[b][1][:]
                rinv = proj.tile([P, NCH], F32, tag=f"rinv{b}")

                for c in range(NCH):
                    W = P * (c + 1)       # causal width for this chunk
                    w0 = S - W            # first needed column of qrel_rev
                    lhs = qT[:, c * P:(c + 1) * P]

                    # qrel_rev chunk -> DRAM bounce (only the needed columns)
                    qr_ps = psB.tile([P, S], F32, tag="qr")
                    nc.tensor.matmul(qr_ps[:, 0:W], r(lhs), r(tableT[:, w0:S]),
                                     start=True, stop=True)
                    qr_sb = work.tile([P, S], F32, tag="qr_sb")
                    nc.vector.tensor_copy(out=qr_sb[:, 0:W], in_=qr_ps[:, 0:W])
                    nc.sync.dma_start(out=qb_ap[c * P:(c + 1) * P, w0:S],
                                      in_=qr_sb[:, 0:W])
                    # rel-shift gather: pos[p, m] = qbuf_flat[767*(128c+p) + 511 + m]
                    # (m > n lands in the NEG pad -> causal mask included)
                    pos_sb = work.tile([P, S], F32, tag="pos")
                    src = bass.AP(tensor=qb_ap.tensor,
                                  offset=qb_base + (PT - 1) * (c * P) + (S - 1),
                                  ap=[[PT - 1, P], [1, W]])
                    nc.sync.dma_start(out=pos_sb[:, 0:W], in_=src)

                    # content chunk (causal-truncated)
                    L_ps = psA.tile([P, S], F32, tag="L")
                    nc.tensor.matmul(L_ps[:, 0:W], r(lhs), r(kT[:, 0:W]),
                                     start=True, stop=True)
                    # logits = content + position (unscaled); scale folded into exp
                    L_sb = work.tile([P, S], F32, tag="L_sb")
                    nc.vector.tensor_add(L_sb[:, 0:W], L_ps[:, 0:W], pos_sb[:, 0:W])
                    mx = work.tile([P, 1], F32, tag="mx")
                    nc.vector.reduce_max(out=mx[:], in_=L_sb[:, 0:W],
                                         axis=mybir.AxisListType.X)
                    negmx = work.tile([P, 1], F32, tag="negmx")
                    nc.scalar.mul(negmx[:], mx[:], -SCALE)
                    E_sb = work.tile([P, S], F32R, tag="E")
                    nc.scalar.activation(out=E_sb[:, 0:W], in_=L_sb[:, 0:W],
                                         func=mybir.ActivationFunctionType.Exp,
                                         bias=negmx[:], scale=SCALE)
                    rowsum = work.tile([P, 1], F32, tag="rowsum")
                    nc.vector.reduce_sum(out=rowsum[:], in_=E_sb[:, 0:W],
                                         axis=mybir.AxisListType.X)
                    nc.vector.reciprocal(out=rinv[:, c:c + 1], in_=rowsum[:])

                    # transpose E chunk row-block: ET(c,j) for j<=c, one PSUM bank
                    et_ps = psA.tile([P, NCH, P], F32, tag="tr")
                    for j in range(c + 1):
                        nc.tensor.transpose(r(et_ps[:, j, :]),
                                            r(E_sb[:, j * P:(j + 1) * P]), r(ident[:]))
                    et_sb = work.tile([P, NCH, P], F32R, tag="et_sb")
                    nc.vector.tensor_copy(out=et_sb[:, 0:c + 1, :],
                                          in_=et_ps[:, 0:c + 1, :])

                    # ctx chunk: sum_j v_j^T.T @ ET(c,j)  -> [64, 128]
                    ctx_ps = psB.tile([DQK, P], F32, tag="ctx")
                    for j in range(c + 1):
                        nc.tensor.matmul(ctx_ps[:], r(v_sb[b][:, j, :]),
                                         r(et_sb[:, j, :]),
                                         start=(j == 0), stop=(j == c))
                    ctx_sb = work.tile([DQK, P], F32R, tag="ctx_sb")
                    nc.vector.tensor_copy(out=ctx_sb[:], in_=ctx_ps[:])

                    # out chunk: (ctx^T @ woT) * rinv, scale fused into the copy
                    o_ps = psB.tile([P, S], F32, tag="out")
                    nc.tensor.matmul(o_ps[:], r(ctx_sb[:]), r(wo_sb[:]),
                                     start=True, stop=True)
                    o_sb = outp.tile([P, S], F32, tag="o_sb")
                    nc.scalar.activation(out=o_sb[:], in_=o_ps[:],
                                         func=mybir.ActivationFunctionType.Copy,
                                         scale=rinv[:, c:c + 1])
                    nc.sync.dma_start(out=out_ext[c * P:(c + 1) * P, b, :],
                                        in_=o_sb[:])

    nc.compile()
    return nc


def _sincos_rev_T():
    """sincosT with reversed j' columns: sct[b_, u] = sincos[1022 - u, b_]."""
    dmin = -(S - 1)
    r_ = (np.arange(2 * S - 1, dtype=np.float32) + np.float32(dmin))
    inv_freq = (1.0 / (10000.0 ** (np.arange(0, D, 2, dtype=np.float32) / np.float32(D)))).astype(np.float32)
    phases = r_[:, None] * inv_freq[None, :]
    sincos = np.concatenate([np.sin(phases), np.cos(phases)], axis=-1).astype(np.float32)  # [1023, 512]
    # j' = n-m in [0, 511] -> row 511 + j'; reversed: u -> row 511 + (511-u) = 1022-u
    sc = sincos[1022 - np.arange(S)]          # [u, b_]
    return np.ascontiguousarray(sc.T)         # [b_, u] = [512, 512]


def _prep_in_maps(x_q, to_q, to_k, to_v, to_out, for_pos_enc):
    xT = np.ascontiguousarray(np.transpose(x_q, (1, 2, 0)).astype(np.float32))  # [B, D, S]
    sct = _sincos_rev_T()
    ident = np.eye(P, dtype=np.float32)
    in_maps = []
    for h in range(H):
        in_maps.append({
            "xT": xT,
            "wqk": np.ascontiguousarray(
                np.concatenate([to_q[:, h, :].T, to_k[:, h, :].T], axis=1).astype(np.float32)),
            "wv": np.ascontiguousarray(to_v[:, h, :].T.astype(np.float32)),
            "pp": np.ascontiguousarray(for_pos_enc[:, h, :].T.astype(np.float32)),
            "wo": np.ascontiguousarray(to_out[:, :, h].T.astype(np.float32)),
            "sincosT": sct,
            "ident": ident,
        })
    return in_maps


def _get_nc():
    if "nc" not in _CACHE:
        _CACHE["nc"] = _build_graph()
    return _CACHE["nc"]


def run(inputs, trace=False, **kw):
    from concourse.bass_utils import run_bass_kernel_spmd
    nc = _get_nc()
    in_maps = _prep_in_maps(**inputs)
    res = run_bass_kernel_spmd(nc, in_maps, core_ids=list(range(H)), trace=trace, **kw)
    out = np.zeros((S, B, D), dtype=np.float32)
    for rr in res.results:
        out += rr["out"]
    return out, res


def kernel(x_q, to_q, to_k, to_v, to_out, for_pos_enc):
    out, _ = run(dict(x_q=x_q, to_q=to_q, to_k=to_k, to_v=to_v,
                      to_out=to_out, for_pos_enc=for_pos_enc))
    return out


# revision 14
# speedup vs baseline: 1.0555x; 1.0555x over previous
"""Trainium2 Bass kernel for Transformer-XL style relative-position attention.

Problem: S=512, B=2, D=512, DQK=64, H=8, causal, OFFSET=0.
Sharding: one head per NeuronCore (8 heads / 8 cores); each core computes its
head's contribution to the output projection; host sums the 8 partials
(row-parallel tensor-parallel unshard).

Per core (head h), per batch b and 128-row query chunk c (causal-truncated to
W = 128*(c+1) key columns):
  qT,kT = Wqk @ x_b^T                  f32r  [64,512] (merged stationary)
  vT    = Wv @ x_b^T -> PE-transposed to bf16 v [m,64] tiles
  tableT_rev = P_h @ sincos_rev^T      f32r  [64,512] rel-pos table (reversed)
  qrel_rev chunk = q @ tableT_rev[:, 512-W:] -> DRAM bounce, row pitch 768,
      pad cols [512,640) pre-filled with -1e30
  position = strided rel-shift gather pos[p,m] = qbuf[767*(128c+p) + 511 + m]
      (m > n lands in the NEG pad -> causal mask for free)
  logits = content + position (DVE add); stable softmax: DVE row-max,
      ScalarE exp (scale folded) with accum_out row-sums
  E (bf16) transposed 128x128 on PE; ctx chunk = sum_j v_j^T @ ET(c,j) (bf16)
  out chunk = (ctx^T @ WoT)(bf16 matmul) * rinv (scale fused into PSUM copy)

QK path runs float32r (fp32 bits, fast PE mode, ~1.6e-4 matmul err); value
path runs bf16 (negligible extra error, enables fast weight loads).
"""

import math

import numpy as np

S, B, D = 512, 2, 512
DQK, H = 64, 8
P = 128
NCH = S // P
KT = D // P
PT = 768          # qbuf row pitch
SCALE = 1.0 / math.sqrt(float(D))
NEG = -1e30

_CACHE = {}


def _build_graph():
    import concourse.bass as bass
    import concourse.mybir as mybir
    import concourse.tile as tile
    from concourse import bacc

    F32 = mybir.dt.float32
    F32R = mybir.dt.float32r
    BF16 = mybir.dt.bfloat16

    nc = bacc.Bacc(None, target_bir_lowering=False, debug=True)

    xT_ext = nc.declare_dram_parameter("xT", [B, D, S], F32R, isOutput=False)
    wqk_ext = nc.declare_dram_parameter("wqk", [D, 2 * DQK], F32R, isOutput=False)
    wv_ext = nc.declare_dram_parameter("wv", [D, DQK], F32R, isOutput=False)
    pp_ext = nc.declare_dram_parameter("pp", [D, DQK], F32R, isOutput=False)
    wo_ext = nc.declare_dram_parameter("wo", [DQK, D], BF16, isOutput=False)
    sct_ext = nc.declare_dram_parameter("sincosT", [D, S], F32R, isOutput=False)
    id_ext = nc.declare_dram_parameter("ident", [P, P], BF16, isOutput=False)
    out_ext = nc.declare_dram_parameter("out", [S, B, D], F32, isOutput=True)

    with tile.TileContext(nc) as tc:
        with tc.tile_pool(name="const", bufs=1) as const, \
             tc.tile_pool(name="proj", bufs=1) as proj, \
             tc.tile_pool(name="work", bufs=3) as work, \
             tc.tile_pool(name="posp", bufs=B * NCH) as posp, \
             tc.tile_pool(name="outp", bufs=2) as outp, \
             tc.tile_pool(name="psA", bufs=2, space="PSUM") as psA, \
             tc.tile_pool(name="psB", bufs=1, space="PSUM") as psB, \
             tc.tile_pool(name="dram", bufs=1, space="DRAM") as dram:

            # ---- input loads (SWDGE; no casts needed, dtypes match) ----
            ident = const.tile([P, P], BF16)
            nc.gpsimd.dma_start(out=ident[:], in_=id_ext[:])
            wqk_sb = const.tile([P, KT, 2 * DQK], F32R)
            nc.gpsimd.dma_start(out=wqk_sb[:], in_=wqk_ext[:].rearrange("(k p) m -> p k m", p=P))
            wv_sb = const.tile([P, KT, DQK], F32R)
            nc.gpsimd.dma_start(out=wv_sb[:], in_=wv_ext[:].rearrange("(k p) m -> p k m", p=P))
            pp_sb = const.tile([P, KT, DQK], F32R)
            nc.gpsimd.dma_start(out=pp_sb[:], in_=pp_ext[:].rearrange("(k p) m -> p k m", p=P))
            wo_sb = const.tile([DQK, S], BF16)
            nc.gpsimd.dma_start(out=wo_sb[:], in_=wo_ext[:])
            sct_sb = const.tile([P, KT, S], F32R)
            nc.gpsimd.dma_start(out=sct_sb[:], in_=sct_ext[:].rearrange("(k p) m -> p k m", p=P))
            xT_sb = []
            for b in range(B):
                xb = const.tile([P, KT, S], F32R, tag=f"xT{b}")
                nc.gpsimd.dma_start(out=xb[:], in_=xT_ext[b].rearrange("(k p) m -> p k m", p=P))
                xT_sb.append(xb)

            # qbuf with NEG pad columns [512, 640)
            qbuf = dram.tile([B, S, PT], F32)
            negt = work.tile([P, P], F32, tag="negt")
            nc.vector.memset(negt[:], NEG)
            for b in range(B):
                for c in range(NCH):
                    nc.gpsimd.dma_start(out=qbuf[b][c * P:(c + 1) * P, S:S + P], in_=negt[:])

            # ---- rel-pos table: tableT_rev [64, 512] f32r ----
            tb_ps = psA.tile([DQK, S], F32, tag="L")
            for k in range(KT):
                nc.tensor.matmul(tb_ps[:], pp_sb[:, k, :], sct_sb[:, k, :],
                                 start=(k == 0), stop=(k == KT - 1))
            tableT = proj.tile([DQK, S], F32R)
            nc.vector.tensor_copy(out=tableT[:], in_=tb_ps[:])

            # ---- projections ----
            qkT = []   # (qx [128,512] f32r rows 0:64=qT, kx [64,512] f32r)
            v_sb = []  # bf16 [128, NCH, 64] m-tiles of v
            for b in range(B):
                qk_ps = psA.tile([P, S], F32, tag="L")
                for k in range(KT):
                    nc.tensor.matmul(qk_ps[:], wqk_sb[:, k, :], xT_sb[b][:, k, :],
                                     start=(k == 0), stop=(k == KT - 1))
                qx = proj.tile([P, S], F32R, tag=f"qkT{b}")
                nc.vector.tensor_copy(out=qx[:], in_=qk_ps[:])
                kx = proj.tile([DQK, S], F32R, tag=f"kT{b}")
                nc.gpsimd.dma_start(out=kx[:], in_=qx[DQK:2 * DQK, :])
                qkT.append((qx, kx))

                vT_ps = psA.tile([DQK, S], F32, tag="L")
                for k in range(KT):
                    nc.tensor.matmul(vT_ps[:], wv_sb[:, k, :], xT_sb[b][:, k, :],
                                     start=(k == 0), stop=(k == KT - 1))
                vT = work.tile([DQK, S], BF16, tag="vT")
                nc.vector.tensor_copy(out=vT[:], in_=vT_ps[:])
                vb = proj.tile([P, NCH, DQK], BF16, tag=f"v{b}")
                for j in range(NCH):
                    vt_ps = psB.tile([P, DQK], BF16, tag="tr0")
                    nc.tensor.transpose(vt_ps[:], vT[:, j * P:(j + 1) * P],
                                        ident[0:DQK, 0:DQK])
                    nc.vector.tensor_copy(out=vb[:, j, :], in_=vt_ps[:])
                v_sb.append(vb)

            # ---- phase 1: all qrel chunks -> DRAM bounce ----
            for b in range(B):
                qT = qkT[b][0][0:DQK, :]
                for c in range(NCH):
                    W = P * (c + 1)
                    w0 = S - W
                    qr_ps = psB.tile([P, S], F32, tag="qr")
                    nc.tensor.matmul(qr_ps[:, 0:W], qT[:, c * P:(c + 1) * P],
                                     tableT[:, w0:S], start=True, stop=True)
                    qr_sb = work.tile([P, S], F32, tag="qr_sb")
                    nc.scalar.copy(out=qr_sb[:, 0:W], in_=qr_ps[:, 0:W])
                    nc.sync.dma_start(out=qbuf[b][c * P:(c + 1) * P, w0:S],
                                      in_=qr_sb[:, 0:W])

            # ---- phase 2: gathers on the scalar HWDGE ring ----
            pos_tiles = {}
            for b in range(B):
                qb_ap = qbuf[b]
                for c in range(NCH):
                    W = P * (c + 1)
                    pos_sb = posp.tile([P, S], F32, tag="pos")
                    src = bass.AP(tensor=qb_ap.tensor,
                                  offset=qb_ap.offset + (PT - 1) * (c * P) + (S - 1),
                                  ap=[[PT - 1, P], [1, W]])
                    nc.scalar.dma_start(out=pos_sb[:, 0:W], in_=src)
                    pos_tiles[(b, c)] = pos_sb

            # ---- phase 3: attention per (b, chunk) ----
            for b in range(B):
                qT = qkT[b][0][0:DQK, :]
                kT = qkT[b][1][:]
                rinv = proj.tile([P, NCH], F32, tag=f"rinv{b}")
                for c in range(NCH):
                    W = P * (c + 1)
                    pos_sb = pos_tiles[(b, c)]

                    L_ps = psA.tile([P, S], F32, tag="L")
                    nc.tensor.matmul(L_ps[:, 0:W], qT[:, c * P:(c + 1) * P],
                                     kT[:, 0:W], start=True, stop=True)
                    L_sb = work.tile([P, S], F32, tag="L_sb")
                    nc.vector.tensor_add(L_sb[:, 0:W], L_ps[:, 0:W], pos_sb[:, 0:W])
                    mx = work.tile([P, 1], F32, tag="mx")
                    nc.vector.reduce_max(out=mx[:], in_=L_sb[:, 0:W],
                                         axis=mybir.AxisListType.X)
                    negmx = work.tile([P, 1], F32, tag="negmx")
                    nc.scalar.mul(negmx[:], mx[:], -SCALE)
                    E_sb = work.tile([P, S], BF16, tag="E")
                    rowsum = work.tile([P, 1], F32, tag="rowsum")
                    nc.scalar.activation(out=E_sb[:, 0:W], in_=L_sb[:, 0:W],
                                         func=mybir.ActivationFunctionType.Exp,
                                         bias=negmx[:], scale=SCALE,
                                         accum_out=rowsum[:])
                    nc.vector.reciprocal(out=rinv[:, c:c + 1], in_=rowsum[:])

                    et_ps = psA.tile([P, NCH, P], BF16, tag="tr")
                    for j in range(c + 1):
                        nc.tensor.transpose(et_ps[:, j, :],
                                            E_sb[:, j * P:(j + 1) * P], ident[:])
                    et_sb = work.tile([P, NCH, P], BF16, tag="et_sb")
                    nc.vector.tensor_copy(out=et_sb[:, 0:c + 1, :],
                                          in_=et_ps[:, 0:c + 1, :])

                    ctx_ps = psB.tile([DQK, P], F32, tag="ctx")
                    for j in range(c + 1):
                        nc.tensor.matmul(ctx_ps[:], v_sb[b][:, j, :], et_sb[:, j, :],
                                         start=(j == 0), stop=(j == c))
                    ctx_sb = work.tile([DQK, P], BF16, tag="ctx_sb")
                    nc.vector.tensor_copy(out=ctx_sb[:], in_=ctx_ps[:])

                    o_ps = psB.tile([P, S], F32, tag="out")
                    nc.tensor.matmul(o_ps[:], ctx_sb[:], wo_sb[:], start=True, stop=True)
                    o_sb = outp.tile([P, S], F32, tag="o_sb")
                    nc.scalar.activation(out=o_sb[:], in_=o_ps[:],
                                         func=mybir.ActivationFunctionType.Copy,
                                         scale=rinv[:, c:c + 1])
                    nc.gpsimd.dma_start(out=out_ext[c * P:(c + 1) * P, b, :],
                                        in_=o_sb[:])

    nc.compile()
    return nc


def _sincos_rev_T():
    """sincosT with reversed j' columns: sct[b_, u] = sincos[1022 - u, b_]."""
    dmin = -(S - 1)
    r_ = (np.arange(2 * S - 1, dtype=np.float32) + np.float32(dmin))
    inv_freq = (1.0 / (10000.0 ** (np.arange(0, D, 2, dtype=np.float32) / np.float32(D)))).astype(np.float32)
    phases = r_[:, None] * inv_freq[None, :]
    sincos = np.concatenate([np.sin(phases), np.cos(phases)], axis=-1).astype(np.float32)
    sc = sincos[1022 - np.arange(S)]
    return np.ascontiguousarray(sc.T)


def _prep_in_maps(x_q, to_q, to_k, to_v, to_out, for_pos_enc):
    import ml_dtypes
    xT = np.ascontiguousarray(np.transpose(x_q, (1, 2, 0)).astype(np.float32))
    sct = _sincos_rev_T()
    ident = np.eye(P, dtype=ml_dtypes.bfloat16)
    in_maps = []
    for h in range(H):
        in_maps.append({
            "xT": xT,
            "wqk": np.ascontiguousarray(
                np.concatenate([to_q[:, h, :].T, to_k[:, h, :].T], axis=1).astype(np.float32)),
            "wv": np.ascontiguousarray(to_v[:, h, :].T.astype(np.float32)),
            "pp": np.ascontiguousarray(for_pos_enc[:, h, :].T.astype(np.float32)),
            "wo": np.ascontiguousarray(to_out[:, :, h].T).astype(ml_dtypes.bfloat16),
            "sincosT": sct,
            "ident": ident,
        })
    return in_maps


def _get_nc():
    if "nc" not in _CACHE:
        _CACHE["nc"] = _build_graph()
    return _CACHE["nc"]


def run(inputs, trace=False, **kw):
    from concourse.bass_utils import run_bass_kernel_spmd
    nc = _get_nc()
    in_maps = _prep_in_maps(**inputs)
    res = run_bass_kernel_spmd(nc, in_maps, core_ids=list(range(H)), trace=trace, **kw)
    out = np.zeros((S, B, D), dtype=np.float32)
    for rr in res.results:
        out += rr["out"]
    return out, res


def kernel(x_q, to_q, to_k, to_v, to_out, for_pos_enc):
    out, _ = run(dict(x_q=x_q, to_q=to_q, to_k=to_k, to_v=to_v,
                      to_out=to_out, for_pos_enc=for_pos_enc))
    return out
